# revision 3
# baseline (speedup 1.0000x reference)
"""Trainium2 Bass kernel for a 3-layer RGCN (mean-aggr per relation) + pooling MLP.

Strategy (8 NeuronCores, SPMD):
  - Nodes sharded into 8 contiguous ranges (dst-sharding); every edge lives on
    the core that owns its destination, so all scatter-adds are core-local.
  - Aggregate-then-transform: per layer, gather source-node features per edge
    (dma_gather, 256B rows), scatter into per-(node,rel) accumulators via
    PE one-hot matmuls (edges pre-sorted by destination window), then apply
    the tiny relation/root weights densely.  Mean-normalisation is folded
    into the one-hot (rows scaled by 1/deg).
  - Between layers an AllGather rebuilds the replicated node-feature table
    each core gathers from.
  - Graph pooling = one-hot matmul against sorted graph ids; partial per-core
    graph sums are AllReduced, the 2-layer MLP head runs redundantly.

Host-side numpy does only sharding/index prep (edge partitioning by window,
int16 index packing, in-degree reciprocals); all FLOPs of the reference run
on device.
"""
import sys

if "/opt/trn_rl_repo" not in sys.path:
    sys.path.insert(0, "/opt/trn_rl_repo")

import numpy as np

P = 128


def _default_cfg():
    return dict(
        N=100000,   # nodes
        E=3200000,  # edges
        G=256,      # graphs
        IN=8,       # input channels
        HID=64,     # hidden channels
        NPC=12500,  # nodes per core (N / 8)
        C=8,        # cores
        SW=2,       # windows per gather span
    )


def _derive(cfg):
    d = dict(cfg)
    d["NPAD"] = ((d["NPC"] + P - 1) // P) * P
    assert d["NPAD"] * 2 % 256 == 0
    d["NW"] = d["NPAD"] * 2 // 256          # seg windows per core (256 segs each)
    assert d["NPAD"] == d["NW"] * P
    d["SEGS"] = d["NPAD"] * 2
    d["TROWS"] = d["C"] * d["NPAD"]          # replicated table rows
    if "CH_OVERRIDE" in d:
        d["CH_ROWS"] = d["CH_OVERRIDE"]
        d["CHUNKS"] = -(-d["TROWS"] // d["CH_ROWS"])
    else:
        d["CHUNKS"] = max(1, -(-d["TROWS"] // 25088))
        d["CH_ROWS"] = -(-d["TROWS"] // d["CHUNKS"])
        d["CH_ROWS"] = ((d["CH_ROWS"] + P - 1) // P) * P
    assert d["CH_ROWS"] <= 32767
    assert d["NW"] % d["SW"] == 0
    d["NSPAN"] = d["NW"] // d["SW"]
    return d


def _preprocess(x, edge_index, edge_type, batch, cfg):
    """Partition/sort edges, build per-core packed index arrays + the shared
    static structure (slot layout)."""
    c = cfg
    C, NPC, NPAD, NW, SW, CHUNKS, CH_ROWS = (
        c["C"], c["NPC"], c["NPAD"], c["NW"], c["SW"], c["CHUNKS"], c["CH_ROWS"],
    )
    NKEY = NW * CHUNKS

    src = np.asarray(edge_index[0], dtype=np.int64)
    dst = np.asarray(edge_index[1], dtype=np.int64)
    et = np.asarray(edge_type, dtype=np.int64)
    batch = np.asarray(batch, dtype=np.int64)

    core = dst // NPC
    dstloc = dst - core * NPC
    seg = dstloc * 2 + et                    # [0, 2*NPC)
    w = dstloc // P                          # window id [0, NW)
    segrel = et * P + (dstloc - w * P)       # rel-major col within window [0,256)
    tab = (src // NPC) * NPAD + (src % NPC)  # replicated-table row
    ck = tab // CH_ROWS
    idxloc = tab - ck * CH_ROWS
    # slot-order key: span-major, chunk-major within span, window within chunk
    key = (w // SW) * (CHUNKS * SW) + ck * SW + (w % SW)

    # per-core group counts -> shared structure
    counts = np.zeros((C, NKEY), np.int64)
    for ci in range(C):
        m = core == ci
        counts[ci] = np.bincount(key[m], minlength=NKEY)
    tmax = counts.max(axis=0)
    T = ((tmax + P - 1) // P) * P            # padded slots per key
    off = np.zeros(NKEY + 1, np.int64)
    np.cumsum(T, out=off[1:])
    total = int(off[-1])
    assert total % P == 0

    # per-key -> (window, chunk)
    key_arr = np.arange(NKEY)
    key_w = (key_arr // (CHUNKS * SW)) * SW + key_arr % SW
    key_ck = (key_arr // SW) % CHUNKS

    deg_all = np.zeros((C, c["SEGS"]), np.int64)
    idx_slots = np.zeros((C, total), np.int16)
    segrel_slots = np.full((C, total), 300.0, np.float32)
    recip_slots = np.zeros((C, total), np.float32)

    for ci in range(C):
        m = core == ci
        k_c, segrel_c, idx_c, seg_c = key[m], segrel[m], idxloc[m], seg[m]
        deg = np.bincount(seg_c, minlength=c["SEGS"])
        deg_all[ci] = deg
        recip_seg = 1.0 / np.maximum(deg, 1).astype(np.float32)
        order = np.argsort(k_c, kind="stable")
        k_s = k_c[order]
        # position within group
        gstart = np.zeros(NKEY, np.int64)
        gstart[1:] = np.cumsum(np.bincount(k_s, minlength=NKEY))[:-1]
        pos = np.arange(k_s.size) - gstart[k_s]
        slot = off[k_s] + pos
        idx_slots[ci, slot] = idx_c[order].astype(np.int16)
        segrel_slots[ci, slot] = segrel_c[order].astype(np.float32)
        recip_slots[ci, slot] = recip_seg[seg_c[order]]

    # packed/wrapped arrays per core
    idxw = np.empty((C, P, total // 16), np.int16)
    segrel_strip = np.empty((C, P, total // P), np.float32)
    recip_strip = np.empty((C, P, total // P), np.float32)
    for ci in range(C):
        idxw[ci] = np.tile(idx_slots[ci].reshape(-1, 16).T, (8, 1))
        segrel_strip[ci] = segrel_slots[ci].reshape(-1, P).T
        recip_strip[ci] = recip_slots[ci].reshape(-1, P).T

    # replicated padded x table [TROWS, 64]
    x = np.asarray(x, dtype=np.float32)
    N, IN = x.shape
    x64 = np.zeros((c["TROWS"], c["HID"]), np.float32)
    n_all = np.arange(N)
    tab_all = (n_all // NPC) * NPAD + (n_all % NPC)
    x64[tab_all, :IN] = x

    # per-core row-major local x slice [NPAD, HID]
    ownx = x64.reshape(C, NPAD, c["HID"])

    # per-core local graph ids [P, NW] (pads -> 999)
    batchloc = np.full((C, P, NW), 999.0, np.float32)
    for ci in range(C):
        b = batch[ci * NPC : (ci + 1) * NPC].astype(np.float32)
        bl = np.full(NPAD, 999.0, np.float32)
        bl[: b.size] = b
        batchloc[ci] = bl.reshape(NW, P).T

    iota = np.tile(np.arange(256, dtype=np.float32), (P, 1))

    # static structure: per span -> gather calls; per window -> tile blocks
    spans = []
    for s in range(c["NSPAN"]):
        base_key = s * CHUNKS * SW
        span_base = int(off[base_key])
        span_slots = int(off[base_key + CHUNKS * SW] - span_base)
        calls = []
        CALL_CAP = 1024
        for k in range(CHUNKS):
            kb = base_key + k * SW
            cb = int(off[kb])
            n = int(off[kb + SW] - off[kb])
            for o in range(0, n, CALL_CAP):
                calls.append((k, cb - span_base + o, min(CALL_CAP, n - o)))
        wins = []
        for wi in range(SW):
            wid = s * SW + wi
            blocks = []
            for k in range(CHUNKS):
                kb = base_key + k * SW + wi
                b0 = int(off[kb])
                for bb in range(int(T[kb]) // P):
                    blocks.append(((b0 - span_base) // P + bb, b0 // P + bb))
            wins.append((wid, blocks))
        spans.append(dict(base=span_base, slots=span_slots, calls=calls, wins=wins))

    struct = dict(total=total, spans=spans, T=T, off=off, key_w=key_w, key_ck=key_ck)
    data = dict(
        idxw=idxw, segrel=segrel_strip, recip=recip_strip, x64=x64, ownx=ownx,
        batchloc=batchloc, iota=iota,
    )
    return struct, data


def _build_program(struct, cfg, debug_dump=False):
    from concourse import bacc, tile, mybir
    from concourse.masks import make_identity

    c = cfg
    C, NPAD, NW, HID, IN = c["C"], c["NPAD"], c["NW"], c["HID"], c["IN"]
    TROWS, CH_ROWS, G = c["TROWS"], c["CH_ROWS"], c["G"]
    total = struct["total"]
    f32 = mybir.dt.float32

    nc = bacc.Bacc("TRN2", target_bir_lowering=False, debug=False, num_devices=C,
                   num_swdge_queues=4)

    # ---- I/O ----
    t_x64 = nc.dram_tensor("x64", [TROWS, HID], f32, kind="ExternalInput")
    t_ownx = nc.dram_tensor("ownx", [NPAD, HID], f32, kind="ExternalInput")
    t_idxw = nc.dram_tensor("idxw", [P, total // 16], mybir.dt.int16, kind="ExternalInput")
    t_segrel = nc.dram_tensor("segrel", [P, total // P], f32, kind="ExternalInput")
    t_recip = nc.dram_tensor("recip", [P, total // P], f32, kind="ExternalInput")
    t_iota = nc.dram_tensor("iota", [P, 256], f32, kind="ExternalInput")
    t_batchloc = nc.dram_tensor("batchloc", [P, NW], f32, kind="ExternalInput")
    t_W1 = nc.dram_tensor("W1", [2, IN, HID], f32, kind="ExternalInput")
    t_root1 = nc.dram_tensor("root1", [IN, HID], f32, kind="ExternalInput")
    t_b1 = nc.dram_tensor("b1", [HID, 1], f32, kind="ExternalInput")
    t_W2 = nc.dram_tensor("W2", [2, HID, HID], f32, kind="ExternalInput")
    t_root2 = nc.dram_tensor("root2", [HID, HID], f32, kind="ExternalInput")
    t_b2 = nc.dram_tensor("b2", [HID, 1], f32, kind="ExternalInput")
    t_W3 = nc.dram_tensor("W3", [2, HID, HID], f32, kind="ExternalInput")
    t_root3 = nc.dram_tensor("root3", [HID, HID], f32, kind="ExternalInput")
    t_b3 = nc.dram_tensor("b3", [HID, 1], f32, kind="ExternalInput")
    t_cW1 = nc.dram_tensor("cW1", [HID, HID // 2], f32, kind="ExternalInput")
    t_cb1 = nc.dram_tensor("cb1", [HID // 2, 1], f32, kind="ExternalInput")
    t_cW2 = nc.dram_tensor("cW2", [HID // 2, 1], f32, kind="ExternalInput")
    t_cb2 = nc.dram_tensor("cb2", [1, 1], f32, kind="ExternalInput")
    t_out = nc.dram_tensor("out", [1, G], f32, kind="ExternalOutput")
    if debug_dump:
        t_dbg_h1 = nc.dram_tensor("dbg_h1", [c["NPAD"], HID], f32, kind="ExternalOutput")
        t_dbg_t0 = nc.dram_tensor("dbg_t0", [TROWS, HID], f32, kind="ExternalOutput")
        t_dbg_g = nc.dram_tensor("dbg_g", [HID, G], f32, kind="ExternalOutput")
        t_dbg_h2 = nc.dram_tensor("dbg_h2", [c["NPAD"], HID], f32, kind="ExternalOutput")

    f32r = mybir.dt.float32r
    relu = mybir.ActivationFunctionType.Relu
    ident_fn = mybir.ActivationFunctionType.Identity
    iseq = mybir.AluOpType.is_equal
    mult = mybir.AluOpType.mult

    with tile.TileContext(nc) as tc:
        with (
            tc.tile_pool(name="const", bufs=1) as cpool,
            tc.tile_pool(name="strips", bufs=1) as spool,
            tc.tile_pool(name="hwin", bufs=3) as hpool,
            tc.tile_pool(name="rootio", bufs=3) as rpool2,
            tc.tile_pool(name="msgs", bufs=3) as mpool,
            tc.tile_pool(name="idxs", bufs=2) as ipool,
            tc.tile_pool(name="oh", bufs=8) as opool,
            tc.tile_pool(name="wins", bufs=4) as wpool,
            tc.tile_pool(name="rows", bufs=4) as rpool,
            tc.tile_pool(name="accP", bufs=2, space="PSUM") as accP,
            tc.tile_pool(name="hP", bufs=2, space="PSUM") as hP,
            tc.tile_pool(name="tP", bufs=3, space="PSUM") as tP,
            tc.tile_pool(name="gP", bufs=1, space="PSUM") as gP,
            tc.tile_pool(name="dram", bufs=1, space="DRAM") as dpool,
        ):
            # ---- constants / strips ----
            ident = cpool.tile([P, P], f32)
            make_identity(nc, ident[:])
            iota_t = cpool.tile([P, 256], f32)
            nc.sync.dma_start(out=iota_t[:], in_=t_iota[:])
            segrel_t = spool.tile([P, total // P], f32)
            nc.sync.dma_start(out=segrel_t[:], in_=t_segrel[:])
            recip_t = spool.tile([P, total // P], f32)
            nc.sync.dma_start(out=recip_t[:], in_=t_recip[:])
            batchloc_t = cpool.tile([P, NW], f32)
            nc.sync.dma_start(out=batchloc_t[:], in_=t_batchloc[:])

            def load_w(src, shape, name):
                t = cpool.tile(list(shape), f32, name=name, tag=name)
                nc.sync.dma_start(out=t[:], in_=src)
                return t

            W1 = [load_w(t_W1[r], (IN, HID), f"w1_{r}") for r in range(2)]
            root1 = load_w(t_root1[:], (IN, HID), "root1w")
            b1 = load_w(t_b1[:], (HID, 1), "b1w")
            W2 = [load_w(t_W2[r], (HID, HID), f"w2_{r}") for r in range(2)]
            root2 = load_w(t_root2[:], (HID, HID), "root2w")
            b2 = load_w(t_b2[:], (HID, 1), "b2w")
            W3 = [load_w(t_W3[r], (HID, HID), f"w3_{r}") for r in range(2)]
            root3 = load_w(t_root3[:], (HID, HID), "root3w")
            b3 = load_w(t_b3[:], (HID, 1), "b3w")
            cW1 = load_w(t_cW1[:], (HID, HID // 2), "cw1w")
            cb1 = load_w(t_cb1[:], (HID // 2, 1), "cb1w")
            cW2 = load_w(t_cW2[:], (HID // 2, 1), "cw2w")
            cb2 = load_w(t_cb2[:], (1, 1), "cb2w")

            # internal DRAM
            shard = [dpool.tile([NPAD, HID], f32, name=f"shard{i}") for i in range(2)]
            table = [dpool.tile([TROWS, HID], f32, name=f"table{i}") for i in range(2)]
            g_in = dpool.tile([HID, G], f32, tag="gin")
            g_out = dpool.tile([HID, G], f32, tag="gout")

            max_span_slots = max(s["slots"] for s in struct["spans"])

            gacc = None  # layer-3 pooling PSUM accumulator

            def run_layer(layer):
                nonlocal gacc
                if layer == 0:
                    gather_src, Wr, rootW, bias, kin = (t_x64, W1, root1, b1, IN)
                elif layer == 1:
                    gather_src, Wr, rootW, bias, kin = (
                        table[0], W2, root2, b2, HID)
                else:
                    gather_src, Wr, rootW, bias, kin = (
                        table[1], W3, root3, b3, HID)
                act = relu if layer < 2 else ident_fn

                def root_rows(wid):
                    lo, hi = wid * P, (wid + 1) * P
                    if layer == 0:
                        return t_ownx[lo:hi, :]
                    return shard[layer - 1][:][lo:hi, :]
                if layer == 2:
                    gacc = gP.tile([HID, 256], f32, space="PSUM")
                first_pool_mm = [True]

                for sp in struct["spans"]:
                    nblk_span = sp["slots"] // P
                    msgs = mpool.tile([P, nblk_span, HID], f32r, tag="msgs")
                    idxt = ipool.tile([P, max_span_slots // 16], mybir.dt.int16,
                                      tag="idxs")
                    col0 = sp["base"] // 16
                    ncol = sp["slots"] // 16
                    nc.sync.dma_start(out=idxt[:, :ncol],
                                      in_=t_idxw[:, col0 : col0 + ncol])
                    for ci_, (k, boff, n) in enumerate(sp["calls"]):
                        hi = min((k + 1) * CH_ROWS, TROWS)
                        src_ap = gather_src[k * CH_ROWS : hi, :] \
                            if layer == 0 else \
                            gather_src[:][k * CH_ROWS : hi, :]
                        nc.gpsimd.dma_gather(
                            out_ap=msgs[:, boff // P : boff // P + n // P, :],
                            in_ap=src_ap.bitcast(f32r),
                            idxs_ap=idxt[:, boff // 16 : (boff + n) // 16],
                            num_idxs=n,
                            num_idxs_reg=n,
                            elem_size=HID,
                            queue_num=ci_ % 4,
                        )
                    for (wid, blocks) in sp["wins"]:
                        acc = accP.tile([kin, 256], f32, space="PSUM", tag="acc")
                        for i, (lb, gb) in enumerate(blocks):
                            oh = opool.tile([P, 256], f32r, tag="oh")
                            nc.any.tensor_scalar(
                                out=oh[:], in0=iota_t[:],
                                scalar1=segrel_t[:, gb : gb + 1],
                                scalar2=recip_t[:, gb : gb + 1],
                                op0=iseq, op1=mult,
                            )
                            nc.tensor.matmul(
                                out=acc[:],
                                lhsT=msgs[:, lb, :kin], rhs=oh[:],
                                start=(i == 0), stop=(i == len(blocks) - 1),
                            )
                        accS = wpool.tile([kin, 256], f32, tag="accS")
                        nc.vector.tensor_copy(out=accS[:], in_=acc[:])
                        # previous-layer features for the root term: load own
                        # row-major rows, transpose to channel-major
                        rootrow = rpool2.tile([P, HID], f32, tag="rootrow")
                        nc.sync.dma_start(out=rootrow[:], in_=root_rows(wid))
                        rtp = tP.tile([HID, P], f32, space="PSUM", tag="tpsum")
                        nc.tensor.transpose(out=rtp[:], in_=rootrow[:],
                                            identity=ident[:])
                        rootT = rpool2.tile([HID, P], f32, tag="rootT")
                        nc.vector.tensor_copy(out=rootT[:], in_=rtp[:])
                        hpsum = hP.tile([HID, P], f32, space="PSUM", tag="hpsum")
                        for r in range(2):
                            nc.tensor.matmul(
                                out=hpsum[:], lhsT=Wr[r][:],
                                rhs=accS[:, r * P : (r + 1) * P],
                                start=(r == 0), stop=False,
                            )
                        nc.tensor.matmul(
                            out=hpsum[:], lhsT=rootW[:],
                            rhs=rootT[:kin, :],
                            start=False, stop=True,
                        )
                        hwin = hpool.tile([HID, P], f32, tag="hwin")
                        nc.scalar.activation(
                            out=hwin[:], in_=hpsum[:], func=act, bias=bias[:],
                        )
                        # transpose to row-major for table shard / pooling
                        tpsum = tP.tile([P, HID], f32, space="PSUM", tag="tpsum")
                        nc.tensor.transpose(
                            out=tpsum[:],
                            in_=hwin[:],
                            identity=ident[:HID, :HID],
                        )
                        rowt = rpool.tile([P, HID], f32, tag="rowt")
                        nc.any.tensor_copy(out=rowt[:], in_=tpsum[:])
                        if layer < 2:
                            nc.sync.dma_start(
                                out=shard[layer][:][wid * P : (wid + 1) * P, :],
                                in_=rowt[:],
                            )
                        else:
                            goh = opool.tile([P, 256], f32, tag="goh")
                            nc.any.tensor_scalar(
                                out=goh[:], in0=iota_t[:],
                                scalar1=batchloc_t[:, wid : wid + 1],
                                scalar2=None, op0=iseq,
                            )
                            nc.tensor.matmul(
                                out=gacc[:],
                                lhsT=rowt[:], rhs=goh[:],
                                start=first_pool_mm[0], stop=(wid == NW - 1),
                            )
                            first_pool_mm[0] = False

            run_layer(0)
            nc.gpsimd.collective_compute(
                "AllGather", mybir.AluOpType.bypass,
                replica_groups=[list(range(C))],
                ins=[shard[0].opt()], outs=[table[0].opt()],
            )
            run_layer(1)
            nc.gpsimd.collective_compute(
                "AllGather", mybir.AluOpType.bypass,
                replica_groups=[list(range(C))],
                ins=[shard[1].opt()], outs=[table[1].opt()],
            )
            run_layer(2)

            # ---- pooling reduce + MLP head ----
            gS = wpool.tile([HID, G], f32, tag="gS")
            nc.any.tensor_copy(out=gS[:], in_=gacc[:, :G])
            nc.sync.dma_start(out=g_in[:], in_=gS[:])
            nc.gpsimd.collective_compute(
                "AllReduce", mybir.AluOpType.add,
                replica_groups=[list(range(C))],
                ins=[g_in.opt()], outs=[g_out.opt()],
            )
            if debug_dump:
                nc.gpsimd.dma_start(out=t_dbg_h1[:], in_=shard[0][:])
                nc.gpsimd.dma_start(out=t_dbg_h2[:], in_=shard[1][:])
                nc.gpsimd.dma_start(out=t_dbg_t0[:], in_=table[0][:])
                nc.gpsimd.dma_start(out=t_dbg_g[:], in_=g_in[:])
            gF = wpool.tile([HID, G], f32, tag="gF")
            nc.sync.dma_start(out=gF[:], in_=g_out[:])
            y1p = hP.tile([HID // 2, G], f32, space="PSUM", tag="hpsum")
            nc.tensor.matmul(out=y1p[:], lhsT=cW1[:], rhs=gF[:],
                             start=True, stop=True)
            y1 = wpool.tile([HID // 2, G], f32, tag="y1")
            nc.scalar.activation(out=y1[:], in_=y1p[:], func=relu, bias=cb1[:])
            y2p = tP.tile([1, G], f32, space="PSUM", tag="tpsum")
            nc.tensor.matmul(out=y2p[:], lhsT=cW2[:], rhs=y1[:],
                             start=True, stop=True)
            y2 = wpool.tile([1, G], f32, tag="y2")
            nc.scalar.activation(out=y2[:], in_=y2p[:], func=ident_fn,
                                 bias=cb2[:])
            nc.sync.dma_start(out=t_out[:], in_=y2[:])

    nc.compile()
    return nc


_CACHE = {}


def _prepare(inputs, cfg, debug_dump=False):
    c = _derive(cfg)
    struct, data = _preprocess(
        inputs["x"], inputs["edge_index"], inputs["edge_type"], inputs["batch"], c
    )
    ckey = (c["N"], c["E"], struct["total"], debug_dump,
            tuple(int(t) for t in struct["T"][:64]))
    if ckey in _CACHE:
        nc = _CACHE[ckey]
    else:
        nc = _build_program(struct, c, debug_dump=debug_dump)
        _CACHE.clear()
        _CACHE[ckey] = nc

    f32 = np.float32
    shared = dict(
        x64=data["x64"], iota=data["iota"],
        W1=np.asarray(inputs["W1"], f32), root1=np.asarray(inputs["root1"], f32),
        b1=np.asarray(inputs["b1"], f32).reshape(-1, 1),
        W2=np.asarray(inputs["W2"], f32), root2=np.asarray(inputs["root2"], f32),
        b2=np.asarray(inputs["b2"], f32).reshape(-1, 1),
        W3=np.asarray(inputs["W3"], f32), root3=np.asarray(inputs["root3"], f32),
        b3=np.asarray(inputs["b3"], f32).reshape(-1, 1),
        cW1=np.asarray(inputs["cW1"], f32),
        cb1=np.asarray(inputs["cb1"], f32).reshape(-1, 1),
        cW2=np.asarray(inputs["cW2"], f32),
        cb2=np.asarray(inputs["cb2"], f32).reshape(-1, 1),
    )
    in_maps = []
    for ci in range(c["C"]):
        m = dict(shared)
        m["ownx"] = np.ascontiguousarray(data["ownx"][ci])
        m["idxw"] = np.ascontiguousarray(data["idxw"][ci])
        m["segrel"] = np.ascontiguousarray(data["segrel"][ci])
        m["recip"] = np.ascontiguousarray(data["recip"][ci])
        m["batchloc"] = np.ascontiguousarray(data["batchloc"][ci])
        in_maps.append(m)
    return nc, in_maps, c


def _run(inputs, cfg, trace=False, debug_dump=False):
    from concourse.bass_utils import run_bass_kernel_spmd

    nc, in_maps, c = _prepare(inputs, cfg, debug_dump=debug_dump)
    res = run_bass_kernel_spmd(nc, in_maps, core_ids=list(range(c["C"])),
                               trace=trace)
    out = res.results[0]["out"].reshape(c["G"], 1).astype(np.float32)
    return out, res


def kernel(**inputs):
    out, _ = _run(inputs, _default_cfg())
    return out

